# revision 30
# baseline (speedup 1.0000x reference)
"""Trainium2 Bass kernel for windowed (block-diagonal) multi-head attention.

Problem nn_Attention_17059610099953:
  x: (8, 1936, 384) tokens of a (B=2, t=4, H=44, W=44) volume; 10x10 spatial
  windows (padded to 50x50 -> 5x5 grid), each window = t*10*10 = 400 tokens of
  12-head attention (head_dim 32), followed by an output projection.

Sharding: 50 windows = 32 full (400 real tokens) + 16 edge (160) + 2 corner
(64). Each of the 8 NeuronCores processes 4 full windows + up to 3
edge/corner windows (compacted + padded to 256 tokens; the softmax
denominator is corrected by +144 to match the reference's 400-slot windows,
whose zero padding tokens each contribute exp(0)=1).

v2: QKV/scores use fp32r for the projections and bf16 for the attention
matmuls (Q/K/V/es/proj operands); the PV matmul runs as column-tiled PAIRS
of heads (M=33 incl. the ones-augmented denominator row) at PE column
offsets 0 and 64, doubling PV throughput. The softmax exp runs on the
Scalar engine with the attention scale folded in; 1/den uses the 1-instr
custom-DVE approx reciprocal; the per-query reciprocal is broadcast across
partitions via a DRAM bounce; the normalize multiplies write the proj
stream in a head-permuted channel order matched by a host-side permutation
of the proj weight rows.
"""
import os
import sys

for _p in ("/opt/trn_rl_repo",):
    if os.path.isdir(_p) and _p not in sys.path:
        sys.path.append(_p)

import numpy as np
import ml_dtypes

import concourse.bass as bass
import concourse.bacc as bacc
import concourse.mybir as mybir
import concourse.tile as tile

F32 = mybir.dt.float32
F32R = mybir.dt.float32r
BF16 = mybir.dt.bfloat16
AF = mybir.ActivationFunctionType

C = 384
NH = 12
HD = 32
SCALE = HD ** -0.5
NF = 4      # full windows per core (n=400)
NS = 3      # small windows per core (n=256)
NFull = 400
NSmall = 256
SMALL_NS = [160, 160, 64]   # per-core small slots: 2 edge + 1 corner
VW = 33     # V_aug per-head column stride (32 dims + ones column)


def ceil_div(a, b):
    return (a + b - 1) // b


# ot channel layout: proj chunk kk, 32-row group a holds head PERM_HEADS[a]
# of the chunk's 4 heads {4kk..4kk+3}; host permutes proj_w rows to match.
PAIR_HEAD_OF_GROUP = [0, 2, 1, 3]  # a -> head offset within chunk


def build_kernel():
    nc = bacc.Bacc("TRN2", target_bir_lowering=False, debug=False, num_devices=8)

    xf = nc.declare_dram_parameter("xf", [NF, 128, 3, NFull], BF16, isOutput=False)
    xs = nc.declare_dram_parameter("xs", [NS, 128, 3, NSmall], BF16, isOutput=False)
    wq = nc.declare_dram_parameter("wq", [128, 3, C], BF16, isOutput=False)
    wk = nc.declare_dram_parameter("wk", [128, 3, C], BF16, isOutput=False)
    wv = nc.declare_dram_parameter("wv", [128, 3, C], BF16, isOutput=False)
    wp = nc.declare_dram_parameter("wp", [128, 3, C], BF16, isOutput=False)
    pb = nc.declare_dram_parameter("pb", [128, 3], F32, isOutput=False)
    zf = nc.declare_dram_parameter("zf", [NF, 128, 3, NFull], F32, isOutput=True)
    zs = nc.declare_dram_parameter("zs", [NS, 128, 3, NSmall], F32, isOutput=True)

    fslots = [(s, NFull, xf, zf, s) for s in range(NF)]
    sslots = [(NF + s, SMALL_NS[s], xs, zs, s) for s in range(NS)]
    slots = [fslots[0], sslots[0], fslots[1], sslots[1],
             fslots[2], fslots[3], sslots[2]]

    with tile.TileContext(nc) as tc:
        with tc.tile_pool(name="weights", bufs=1) as wpool, \
             tc.tile_pool(name="xio", bufs=2) as xpool, \
             tc.tile_pool(name="qk", bufs=2) as qkpool, \
             tc.tile_pool(name="vaug", bufs=2) as vpool, \
             tc.tile_pool(name="es", bufs=3) as espool, \
             tc.tile_pool(name="oz", bufs=2) as ozpool, \
             tc.tile_pool(name="nrm", bufs=4) as nrmpool, \
             tc.tile_pool(name="nrmbig", bufs=2) as nbpool, \
             tc.tile_pool(name="dscratch", bufs=6, space="DRAM") as dpool, \
             tc.tile_pool(name="ps_s", bufs=2, space="PSUM") as ps_s, \
             tc.tile_pool(name="ps_pv", bufs=2, space="PSUM") as ps_pv, \
             tc.tile_pool(name="ps_mm", bufs=2, space="PSUM") as ps_mm:

            twq = wpool.tile([128, 3, C], BF16, tag="wq")
            twk = wpool.tile([128, 3, C], BF16, tag="wk")
            twv = wpool.tile([128, 3, C], BF16, tag="wv")
            twp = wpool.tile([128, 3, C], BF16, tag="wp")
            tpb = wpool.tile([128, 3], F32, tag="pb")
            nc.gpsimd.dma_start(out=twq[:], in_=wq[:])
            nc.gpsimd.dma_start(out=twk[:], in_=wk[:])
            nc.scalar.dma_start(out=twv[:], in_=wv[:])
            nc.scalar.dma_start(out=twp[:], in_=wp[:])
            nc.scalar.dma_start(out=tpb[:], in_=pb[:])

            # HAM warm-up: ~4us of dummy matmuls run during the input DMA
            # preamble (results unused) so the PE clock gate opens to 8/8
            # before the first real matmul instead of ~3.4us into it
            wrm = wpool.tile([128, 512], BF16, tag="wrm")
            nc.vector.memset(wrm[:], 1.0)
            pwk = ps_mm.tile([128, 512], F32, tag="mm")
            for r in range(14):
                nc.tensor.matmul(pwk[:, 0:400], wrm[:, 0:128], wrm[:, 0:400],
                                 start=(r == 0), stop=(r == 13))

            pending = None
            for slot, n, xin, zout, si in slots:
                n_mt = ceil_div(n, 128)
                m_sizes = [min(128, n - 128 * j) for j in range(n_mt)]

                xt = xpool.tile([128, 3, NFull], BF16, tag="xt")
                nc.sync.dma_start(out=xt[:, :, 0:n], in_=xin[si][:, :, 0:n])

                qt = qkpool.tile([128, 3, NFull], BF16, tag="qt")
                kt = qkpool.tile([128, 3, NFull], BF16, tag="kt")
                vg = vpool.tile([128, n_mt, NH * VW], BF16, tag="vg")

                def emit_qk_group(dst, w, i, n=n, xt=xt):
                    pmm = ps_mm.tile([128, 512], F32, tag="mm")
                    for kk in range(3):
                        nc.tensor.matmul(pmm[:, 0:n], w[:, kk, 128 * i:128 * i + 128],
                                         xt[:, kk, 0:n], start=(kk == 0), stop=(kk == 2))
                    nc.vector.tensor_copy(dst[:, i, 0:n], pmm[:, 0:n])

                def emit_v_group(j, m_sizes=m_sizes, xt=xt, vg=vg):
                    mj = m_sizes[j]
                    pmm = ps_mm.tile([128, 512], F32, tag="mm")
                    for kk in range(3):
                        nc.tensor.matmul(pmm[0:mj, 0:C], xt[:, kk, 128 * j:128 * j + mj],
                                         twv[:, kk, :], start=(kk == 0), stop=(kk == 2))
                    vslice = vg[0:mj, j, 0:NH * VW].rearrange("p (h c) -> p h c", h=NH)
                    nc.vector.tensor_copy(vslice[:, :, 0:32],
                                          pmm[0:mj, 0:C].rearrange("p (h c) -> p h c", h=NH))
                    nc.vector.memset(vslice[:, :, 32:33], 1.0)

                emit_qk_group(qt, twq, 0)
                emit_qk_group(kt, twk, 0)
                vq = list(range(n_mt))

                # unnormalized PV output for all 6 head-pairs; pair p: head 2p
                # at rows 0:32 (den row 32), head 2p+1 at rows 64:96 (den 96)
                oun = nbpool.tile([128, 6, NFull], BF16, tag="oun")

                def emit_pv(p, esA, esB, n=n, n_mt=n_mt, m_sizes=m_sizes, vg=vg,
                            oun=oun):
                    hA, hB = 2 * p, 2 * p + 1
                    ppv = ps_pv.tile([128, 512], F32, tag="pv")
                    for j in range(n_mt):
                        mj = m_sizes[j]
                        nc.tensor.matmul(ppv[0:33, 0:n],
                                         vg[0:mj, j, VW * hA:VW * hA + 33],
                                         esA[0:mj, j, 0:n],
                                         start=(j == 0), stop=(j == n_mt - 1),
                                         tile_position=(0, 0))
                        nc.tensor.matmul(ppv[64:97, 0:n],
                                         vg[0:mj, j, VW * hB:VW * hB + 33],
                                         esB[0:mj, j, 0:n],
                                         start=(j == 0), stop=(j == n_mt - 1),
                                         tile_position=(0, 64))
                    nc.vector.tensor_copy(oun[0:97, p, 0:n], ppv[0:97, 0:n])

                pend_pv = None
                for p in range(6):
                    hA, hB = 2 * p, 2 * p + 1
                    tiA, toA = hA // 4, 32 * (hA % 4)
                    tiB, toB = hB // 4, 32 * (hB % 4)
                    # both heads of the pair share each PSUM tile on distinct
                    # PE row groups, so the two matmuls run concurrently
                    es2 = espool.tile([128, 2, 4, NFull], BF16, tag="es")
                    for j in range(n_mt):
                        mj = m_sizes[j]
                        pss = ps_s.tile([128, 2, 512], F32, tag="s")
                        nc.tensor.matmul(
                            pss[0:mj, 0, 0:n],
                            kt[toA:toA + 32, tiA, 128 * j:128 * j + mj],
                            qt[toA:toA + 32, tiA, 0:n],
                            start=True, stop=True, tile_position=(toA, 0))
                        nc.tensor.matmul(
                            pss[0:mj, 1, 0:n],
                            kt[toB:toB + 32, tiB, 128 * j:128 * j + mj],
                            qt[toB:toB + 32, tiB, 0:n],
                            start=True, stop=True, tile_position=(toB, 0))
                        nc.scalar.activation(
                            es2[:, :, j, 0:n], pss[:, 0:2, 0:n], AF.Exp,
                            scale=SCALE)
                    if p == 0:
                        while vq:
                            emit_v_group(vq.pop(0))
                    elif p == 1:
                        emit_qk_group(qt, twq, 1)
                        emit_qk_group(kt, twk, 1)
                    elif p == 3:
                        emit_qk_group(qt, twq, 2)
                        emit_qk_group(kt, twk, 2)
                    if slot == NF + NS - 1 and p >= 1:
                        # last (corner) window: the PE is otherwise idle here
                        # waiting on the F3 normalize chain — keep the clock
                        # gate warm for the two final tail projections
                        pwt = ps_mm.tile([128, 512], F32, tag="mm")
                        for r in range(3):
                            nc.tensor.matmul(pwt[:, 0:400], wrm[:, 0:128],
                                             wrm[:, 0:400],
                                             start=(r == 0), stop=(r == 2))
                    if pend_pv is not None:
                        pend_pv()
                    pend_pv = lambda p=p, es2=es2: emit_pv(p, es2[:, 0], es2[:, 1])
                pend_pv()

                # previous window's tail first: its multiplies/proj must not
                # queue behind this window's normalize DMA chain (FIFO queues)
                if pending is not None:
                    pending()
                    pending = None

                # denominators: rows 32 (even heads) and 96 (odd heads) of oun
                dal = nrmpool.tile([12, NFull], BF16, tag="dal")
                nc.sync.dma_start(out=dal[0:6, 0:n], in_=oun[32:33, 0:6, 0:n])
                nc.sync.dma_start(out=dal[6:12, 0:n], in_=oun[96:97, 0:6, 0:n])
                dfl = nrmpool.tile([12, NFull], F32, tag="dfl")
                # reference windows have 400 token slots; padded-256 windows
                # under-count the zero-token exp(0)=1 terms
                nc.vector.tensor_scalar_add(dfl[:, 0:n], dal[:, 0:n],
                                            float(NFull - n))
                rcp = nrmpool.tile([12, NFull], F32, tag="rcp")
                nc.vector.reciprocal_approx_fast(out=rcp[:, 0:n], in_=dfl[:, 0:n])
                rcb = nrmpool.tile([12, NFull], BF16, tag="rcb")
                nc.vector.tensor_copy(rcb[:, 0:n], rcp[:, 0:n])
                dsc = dpool.tile([12, NFull], BF16, tag="dsc")
                nc.sync.dma_start(out=dsc[:, 0:n], in_=rcb[:, 0:n])
                # broadcast 1/den across 32 partitions, aligned with oun rows
                # (tensor_tensor needs both SBUF inputs at one base partition):
                # rows 0:32 = even heads (pair p at free idx p), 64:96 = odd
                bca = nbpool.tile([128, 6, NFull], BF16, tag="bca")
                nc.sync.dma_start(
                    out=bca[0:32, :, 0:n],
                    in_=dsc[None, 0:6, 0:n].to_broadcast((32, 6, n)))
                nc.sync.dma_start(
                    out=bca[64:96, :, 0:n],
                    in_=dsc[None, 6:12, 0:n].to_broadcast((32, 6, n)))

                def tail(n=n, zout=zout, si=si, oun=oun, bca=bca):
                    ot = ozpool.tile([128, 3, NFull], BF16, tag="ot")
                    for p in range(6):
                        kk, half = p // 2, p % 2
                        for r in range(2):
                            dst = 64 * r + 32 * half
                            eng = nc.vector if p % 2 == 0 else nc.gpsimd
                            eng.tensor_mul(ot[dst:dst + 32, kk, 0:n],
                                           oun[64 * r:64 * r + 32, p, 0:n],
                                           bca[64 * r:64 * r + 32, p, 0:n])
                    zt = ozpool.tile([128, 3, NFull], F32, tag="zt")
                    for i in range(3):
                        pmm = ps_mm.tile([128, 512], F32, tag="mm")
                        for kk in range(3):
                            nc.tensor.matmul(pmm[:, 0:n],
                                             twp[:, kk, 128 * i:128 * i + 128],
                                             ot[:, kk, 0:n],
                                             start=(kk == 0), stop=(kk == 2))
                        nc.vector.tensor_scalar_add(zt[:, i, 0:n], pmm[:, 0:n],
                                                    tpb[:, i:i + 1])
                    nc.sync.dma_start(out=zout[si][:, :, 0:n], in_=zt[:, :, 0:n])

                pending = tail

            if pending is not None:
                pending()

    nc.compile()
    return nc


WS = 10
B, T, H, W = 2, 4, 44, 44
HG = WG = 5


def window_partition(x):
    """x: (B*T, H*W, C) -> windows (B, 25, 400, C) padded, plus metadata."""
    ax = x.reshape(B, T, H, W, C)
    pad = WS * HG
    axp = np.zeros((B, T, pad, pad, C), dtype=x.dtype)
    axp[:, :, :H, :W, :] = ax
    axp = axp.reshape(B, T, HG, WS, WG, WS, C)
    axp = axp.transpose(0, 2, 4, 1, 3, 5, 6).reshape(B, HG * WG, T * WS * WS, C)
    return axp


def classify_windows():
    """Return (full_list, small_list) of (b, w, n_valid)."""
    full, small = [], []
    for b in range(B):
        for i in range(HG):
            for j in range(WG):
                w = i * WG + j
                vi = min(WS, H - i * WS)
                vj = min(WS, W - j * WS)
                nv = T * vi * vj
                if vi == WS and vj == WS:
                    full.append((b, w))
                else:
                    small.append((b, w, nv))
    return full, small


def window_token_index(w):
    """For window w, indices of its 400 token slots ordered by (t, wi, wj),
    and validity mask."""
    i, j = w // WG, w % WG
    idx = np.zeros((T, WS, WS), dtype=np.int64)
    valid = np.zeros((T, WS, WS), dtype=bool)
    for t in range(T):
        for a in range(WS):
            for bb in range(WS):
                hh, ww = i * WS + a, j * WS + bb
                ok = (hh < H) and (ww < W)
                valid[t, a, bb] = ok
                idx[t, a, bb] = (t * H + min(hh, H - 1)) * W + min(ww, W - 1)
    return idx.reshape(-1), valid.reshape(-1)


def compact_window_tokens(xw, w):
    """xw: (400, C) padded window tokens (zeros at invalid). Returns
    (n_valid tokens compacted, order) where order lists the valid slot ids."""
    _, valid = window_token_index(w)
    order = np.nonzero(valid)[0]
    return xw[order], order


def proj_row_perm():
    """Row permutation of proj_w.T matching the kernel's ot channel layout:
    chunk kk row-group a holds head 4kk + PAIR_HEAD_OF_GROUP[a]."""
    perm = np.zeros(C, dtype=np.int64)
    for kk in range(3):
        for a, ho in enumerate(PAIR_HEAD_OF_GROUP):
            h = 4 * kk + ho
            perm[128 * kk + 32 * a:128 * kk + 32 * a + 32] = \
                np.arange(32 * h, 32 * h + 32)
    return perm


def shard_inputs(x, qkv_w, proj_w, proj_b):
    """Build per-core in_maps. Returns (in_maps, meta)."""
    x = np.asarray(x, dtype=np.float32)
    xw = window_partition(x)           # (B, 25, 400, C)
    full, small = classify_windows()
    assert len(full) == 32 and len(small) == 18

    # per-core assignment: 4 full; small slots = [edge, edge, corner] where
    # edges have 160 valid tokens, corners 64 (cores 2-7 get a zero corner)
    edges = [s for s in small if s[2] == 160]
    corners = [s for s in small if s[2] == 64]
    assert len(edges) == 16 and len(corners) == 2
    full_assign = [full[4 * c:4 * c + 4] for c in range(8)]
    small_assign = [edges[2 * c:2 * c + 2] + corners[c:c + 1] for c in range(8)]
    meta = {"full": full_assign, "small": small_assign, "orders": {}}

    wqT = qkv_w[0:C, :].T.astype(np.float32)      # (C, C): [c, qf]
    wkT = qkv_w[C:2 * C, :].T.astype(np.float32)
    wvT = qkv_w[2 * C:3 * C, :].T.astype(np.float32)
    wpT = proj_w.T.astype(np.float32)[proj_row_perm(), :]

    def wtile(wt):  # (C=384 rows c, C cols f) -> [128, 3, 384]
        return np.ascontiguousarray(wt.reshape(3, 128, C).transpose(1, 0, 2))

    in_maps = []
    for c in range(8):
        xfa = np.zeros((NF, 128, 3, NFull), dtype=ml_dtypes.bfloat16)
        for s, (b, w) in enumerate(full_assign[c]):
            xt = xw[b, w].T                      # (C, 400)
            xfa[s] = xt.reshape(3, 128, NFull).transpose(1, 0, 2)
        xsa = np.zeros((NS, 128, 3, NSmall), dtype=ml_dtypes.bfloat16)
        for s, (b, w, nv) in enumerate(small_assign[c]):
            toks, order = compact_window_tokens(xw[b, w], w)
            meta["orders"][(b, w)] = order
            xt = np.zeros((C, NSmall), dtype=np.float32)
            xt[:, 0:nv] = toks.T
            xsa[s] = xt.reshape(3, 128, NSmall).transpose(1, 0, 2)
        in_maps.append({
            "xf": xfa, "xs": xsa,
            "wq": wtile(wqT).astype(ml_dtypes.bfloat16),
            "wk": wtile(wkT).astype(ml_dtypes.bfloat16),
            "wv": wtile(wvT).astype(ml_dtypes.bfloat16),
            "wp": wtile(wpT).astype(ml_dtypes.bfloat16),
            "pb": np.ascontiguousarray(proj_b.astype(np.float32).reshape(3, 128).T),
        })
    return in_maps, meta


def unshard_outputs(results, meta):
    """results: list of 8 dicts with zf (NF,128,3,400), zs. Return (B*T, H*W, C)."""
    zwin = np.zeros((B, HG * WG, T * WS * WS, C), dtype=np.float32)
    for c in range(8):
        zfc, zsc = results[c]["zf"], results[c]["zs"]
        for s, (b, w) in enumerate(meta["full"][c]):
            zt = zfc[s].transpose(1, 0, 2).reshape(C, NFull)   # (C, 400)
            zwin[b, w] = zt.T
        for s, (b, w, nv) in enumerate(meta["small"][c]):
            zt = zsc[s].transpose(1, 0, 2).reshape(C, NSmall)
            order = meta["orders"][(b, w)]
            zwin[b, w][order] = zt.T[0:nv]
    # reverse window partition
    z = zwin.reshape(B, HG, WG, T, WS, WS, C)
    z = z.transpose(0, 3, 1, 4, 2, 5, 6).reshape(B, T, HG * WS, WG * WS, C)
    z = z[:, :, :H, :W, :]
    return z.reshape(B * T, H * W, C)


_CACHE = {}


def _get_nc():
    if "nc" not in _CACHE:
        _CACHE["nc"] = build_kernel()
    return _CACHE["nc"]


def kernel(x, qkv_w, proj_w, proj_b, t=4, H=44, W=44, **_unused):
    from concourse.bass_utils import run_bass_kernel_spmd

    x = np.asarray(x, dtype=np.float32)
    qkv_w = np.asarray(qkv_w, dtype=np.float32)
    proj_w = np.asarray(proj_w, dtype=np.float32)
    proj_b = np.asarray(proj_b, dtype=np.float32)
    in_maps, meta = shard_inputs(x, qkv_w, proj_w, proj_b)
    nc = _get_nc()
    res = run_bass_kernel_spmd(nc, in_maps, list(range(8)))
    return unshard_outputs(res.results, meta)
